# revision 1
# baseline (speedup 1.0000x reference)
"""Trainium2 Bass kernel for nn_Interpolator: zero-stuff upsample x8 + 128-tap FIR (SAME) + x8 gain.

Polyphase formulation: with m indexing 64-sample rows of x and n = 8*q' + r in [0, 512),
    y[512*m + n] = sum_{k=0}^{78} T4[k, m] * H4[k, n]
where T4[k, m] = x[64*m + k - 7] (zero-padded) and
    H4[k, 8*q'+r] = 8 * h[(7-r) + 8*(k-q')]  for 0 <= k-q' <= 15, else 0.

Per core (8 cores, batch-parallel): 16 signals (8 batch rows x {real, imag}).
T4 [128, 512] fp16 is materialized straight from HBM with one xbar DMA-transpose per
signal (src AP = 512 overlapping rows of 128 samples, stride 64 — satisfies the
16-row/128-col xbar tiling and the 2-byte dtype constraint).  Then 4 matmuls
lhsT=T4[0:79, 128t:+128], rhs=H4 [79, 512] fp16 -> PSUM fp32 [128, 512]; PSUM is
cast-copied to fp16 SBUF and stored contiguously (partition i of tile t holds
y[65536t + 512i : +512]).  y is fp16 on device; the host casts to fp32 — halves
store traffic at ~2e-4 added relative error.  Matmuls are batched 4 signals per
stage so the PE sees dense bursts and HAM unthrottles to 2.4 GHz.
"""

import numpy as np

import concourse.bass as bass
import concourse.tile as tile
from concourse import bacc, mybir
from concourse.bass_utils import run_bass_kernel_spmd

B = 64
N = 32768
FACTOR = 8
NOUT = N * FACTOR  # 262144
N_CORES = 8
ROWS_PER_CORE = B // N_CORES  # 8
SIGS = 2 * ROWS_PER_CORE  # 16 signals per core (real rows then imag rows)
K = 79  # contraction window length
NPAD = 32832  # 7 leading zeros + N + 57 trailing zeros (host-padded); 64*511+128 = 32832
TILES = 4  # out tiles per signal, each [128 m-rows, 512 samples]

_F16 = mybir.dt.float16
_F32 = mybir.dt.float32

_NC_CACHE = {}


def _build_nc():
    nc = bacc.Bacc(
        "TRN2",
        target_bir_lowering=False,
        debug=False,
        enable_asserts=False,
        num_devices=N_CORES,
    )
    x = nc.dram_tensor("x", [SIGS, NPAD], _F16, kind="ExternalInput")
    h4 = nc.dram_tensor("h4", [K, 512], _F16, kind="ExternalInput")
    y = nc.dram_tensor("y", [SIGS, NOUT], _F16, kind="ExternalOutput")

    with tile.TileContext(nc) as tc:
        with (
            tc.tile_pool(name="consts", bufs=1) as consts,
            tc.tile_pool(name="t4pool", bufs=8) as t4pool,
            tc.tile_pool(name="opool", bufs=6) as opool,
            tc.tile_pool(name="po", bufs=3, space="PSUM") as po_pool,
        ):
            h4_sb = consts.tile([K, 512], _F16)
            nc.sync.dma_start(out=h4_sb, in_=h4.ap())

            t4_tiles = [None] * SIGS

            def stage_a(sig):
                """One xbar DMA-transpose: T4[k, m] = x_pad[sig, 64m + k]."""
                T4 = t4pool.tile([128, 512], _F16)
                nc.scalar.dma_start(
                    out=T4[:, :],
                    in_=bass.AP(
                        tensor=x, offset=sig * NPAD, ap=[[64, 512], [1, 128]]
                    ),
                    transpose=True,
                )
                t4_tiles[sig] = T4

            def stage_b(sig):
                """Matmuls + cast-copy + store (two independent 256 KB halves)."""
                T4 = t4_tiles[sig]
                out_sb = opool.tile([128, TILES * 512], _F16)
                for half in range(2):
                    po = po_pool.tile([128, 1024], _F32)
                    for s in range(2):
                        t = 2 * half + s
                        nc.tensor.matmul(
                            po[:, 512 * s : 512 * (s + 1)],
                            T4[0:K, 128 * t : 128 * (t + 1)],
                            h4_sb[:, :],
                            start=True,
                            stop=True,
                        )
                    if half == 0:
                        nc.scalar.copy(out=out_sb[:, 0:1024], in_=po)
                    else:
                        nc.vector.tensor_copy(out=out_sb[:, 1024:2048], in_=po)
                    # partition i, free (t, n) -> y[sig, 65536t + 512i + n]
                    nc.sync.dma_start(
                        out=bass.AP(
                            tensor=y,
                            offset=sig * NOUT + half * 2 * 65536,
                            ap=[[512, 128], [65536, 2], [1, 512]],
                        ),
                        in_=out_sb[:, 1024 * half : 1024 * (half + 1)],
                    )

            # Batch 4 signals per stage: 16 back-to-back matmuls per stage_b
            # keep the PE busy long enough for HAM to unthrottle to 2.4 GHz.
            BATCH = 4
            for b in range(SIGS // BATCH):
                for s in range(BATCH):
                    stage_a(BATCH * b + s)
                if b >= 1:
                    for s in range(BATCH):
                        stage_b(BATCH * (b - 1) + s)
            for s in range(BATCH):
                stage_b(SIGS - BATCH + s)

    nc.compile()
    return nc


def _get_nc():
    if "nc" not in _NC_CACHE:
        _NC_CACHE["nc"] = _build_nc()
    return _NC_CACHE["nc"]


def _build_h4(h):
    h4 = np.zeros((K, 512), np.float32)
    qp = np.arange(64)
    for t in range(16):
        for r in range(8):
            h4[qp + t, 8 * qp + r] = FACTOR * h[(7 - r) + 8 * t]
    return h4


def _run(x_real, x_imag, fir_filter, trace=False):
    h4 = _build_h4(np.asarray(fir_filter, np.float32)).astype(np.float16)
    in_maps = []
    for c in range(N_CORES):
        rows = slice(c * ROWS_PER_CORE, (c + 1) * ROWS_PER_CORE)
        shard = np.zeros((SIGS, NPAD), np.float16)
        shard[:ROWS_PER_CORE, 7 : 7 + N] = x_real[rows]
        shard[ROWS_PER_CORE:, 7 : 7 + N] = x_imag[rows]
        in_maps.append({"x": shard, "h4": h4})
    nc = _get_nc()
    res = run_bass_kernel_spmd(nc, in_maps, core_ids=list(range(N_CORES)), trace=trace)
    out = np.empty((2, B, NOUT), np.float32)
    for c in range(N_CORES):
        yc = res.results[c]["y"]
        rows = slice(c * ROWS_PER_CORE, (c + 1) * ROWS_PER_CORE)
        out[0, rows] = yc[:ROWS_PER_CORE]
        out[1, rows] = yc[ROWS_PER_CORE:]
    return out, res


def kernel(x_real, x_imag, fir_filter, factor):
    assert int(factor) == FACTOR
    x_real = np.asarray(x_real, np.float32)
    x_imag = np.asarray(x_imag, np.float32)
    assert x_real.shape == (B, N) and x_imag.shape == (B, N)
    out, _ = _run(x_real, x_imag, fir_filter)
    return out



# revision 2
# speedup vs baseline: 1.2826x; 1.2826x over previous
"""Trainium2 Bass kernel for nn_Interpolator: zero-stuff upsample x8 + 128-tap FIR (SAME) + x8 gain.

Polyphase formulation: with m indexing 64-sample rows of x and n = 8*q' + r in [0, 512),
    y[512*m + n] = sum_{k=0}^{78} T4[k, m] * H4[k, n]
where T4[k, m] = x[64*m + k - 7] (zero-padded) and
    H4[k, 8*q'+r] = 8 * h[(7-r) + 8*(k-q')]  for 0 <= k-q' <= 15, else 0.

The T4 matrices are built on the HOST (numpy stride tricks) and shipped to the device
pre-transposed, so the device input path is 16 plain contiguous 81 KB loads on the
gpsimd (SWDGE) ring — no xbar DMA-transpose.  Per signal: 4 matmuls
lhsT=T4[0:79, 128t:+128], rhs=H4 [79, 512] fp16 -> PSUM fp32 [128, 512]; PSUM is
cast-copied to fp16 SBUF (alternating scalar/vector so the two PSUM-capable engines
split the 37 us of copy work) and stored with 256 KB DMAs on the sync ring, which
carries stores only.  y is fp16 on device; the host casts to fp32.
"""

import numpy as np

import concourse.bass as bass
import concourse.tile as tile
from concourse import bacc, mybir
from concourse.bass_utils import run_bass_kernel_spmd

B = 64
N = 32768
FACTOR = 8
NOUT = N * FACTOR  # 262144
N_CORES = 8
ROWS_PER_CORE = B // N_CORES  # 8
SIGS = 2 * ROWS_PER_CORE  # 16 signals per core (real rows then imag rows)
K = 79  # contraction window length
M = 512  # 64-sample blocks per signal
NPAD = 32832  # 7 leading zeros + N + 57 trailing zeros
TILES = 4  # out tiles per signal, each [128 m-rows, 512 samples]

_F16 = mybir.dt.float16
_F32 = mybir.dt.float32

_NC_CACHE = {}


def _build_nc():
    nc = bacc.Bacc(
        "TRN2",
        target_bir_lowering=False,
        debug=False,
        enable_asserts=False,
        num_devices=N_CORES,
    )
    xt = nc.dram_tensor("xt", [SIGS * K, M], _F16, kind="ExternalInput")
    h4 = nc.dram_tensor("h4", [K, 512], _F16, kind="ExternalInput")
    y = nc.dram_tensor("y", [SIGS, NOUT], _F16, kind="ExternalOutput")

    with tile.TileContext(nc) as tc:
        with (
            tc.tile_pool(name="consts", bufs=1) as consts,
            tc.tile_pool(name="t4pool", bufs=4) as t4pool,
            tc.tile_pool(name="opool", bufs=6) as opool,
            tc.tile_pool(name="po", bufs=4, space="PSUM") as po_pool,
        ):
            h4_sb = consts.tile([K, 512], _F16)
            nc.gpsimd.dma_start(out=h4_sb, in_=h4.ap())

            t4_tiles = [None] * SIGS

            def load(sig):
                T4 = t4pool.tile([K, M], _F16)
                nc.gpsimd.dma_start(
                    out=T4,
                    in_=bass.AP(tensor=xt, offset=sig * K * M, ap=[[M, K], [1, M]]),
                )
                t4_tiles[sig] = T4

            def compute(sig):
                T4 = t4_tiles[sig]
                for half in range(2):
                    po = po_pool.tile([128, 1024], _F32)
                    for s in range(2):
                        t = 2 * half + s
                        nc.tensor.matmul(
                            po[:, 512 * s : 512 * (s + 1)],
                            T4[0:K, 128 * t : 128 * (t + 1)],
                            h4_sb[:, :],
                            start=True,
                            stop=True,
                        )
                    out_sb = opool.tile([128, 1024], _F16)
                    if half == 0:
                        nc.scalar.copy(out=out_sb, in_=po)
                    else:
                        nc.vector.tensor_copy(out=out_sb, in_=po)
                    # partition i, free (t, n) -> y[sig, 65536t + 512i + n]
                    nc.sync.dma_start(
                        out=bass.AP(
                            tensor=y,
                            offset=sig * NOUT + half * 2 * 65536,
                            ap=[[512, 128], [65536, 2], [1, 512]],
                        ),
                        in_=out_sb,
                    )

            PREF = 3
            for s in range(PREF):
                load(s)
            for sig in range(SIGS):
                if sig + PREF < SIGS:
                    load(sig + PREF)
                compute(sig)

    nc.compile()
    return nc


def _get_nc():
    if "nc" not in _NC_CACHE:
        _NC_CACHE["nc"] = _build_nc()
    return _NC_CACHE["nc"]


def _build_h4(h):
    h4 = np.zeros((K, 512), np.float32)
    qp = np.arange(64)
    for t in range(16):
        for r in range(8):
            h4[qp + t, 8 * qp + r] = FACTOR * h[(7 - r) + 8 * t]
    return h4


def _run(x_real, x_imag, fir_filter, trace=False):
    h4 = _build_h4(np.asarray(fir_filter, np.float32)).astype(np.float16)
    in_maps = []
    for c in range(N_CORES):
        rows = slice(c * ROWS_PER_CORE, (c + 1) * ROWS_PER_CORE)
        xp = np.zeros((SIGS, NPAD), np.float16)
        xp[:ROWS_PER_CORE, 7 : 7 + N] = x_real[rows]
        xp[ROWS_PER_CORE:, 7 : 7 + N] = x_imag[rows]
        # T4[sig, k, m] = xp[sig, 64*m + k] -- host-side transpose
        v = np.lib.stride_tricks.as_strided(
            xp,
            shape=(SIGS, K, M),
            strides=(xp.strides[1] * NPAD, xp.strides[1], 64 * xp.strides[1]),
        )
        xt = np.ascontiguousarray(v).reshape(SIGS * K, M)
        in_maps.append({"xt": xt, "h4": h4})
    nc = _get_nc()
    res = run_bass_kernel_spmd(nc, in_maps, core_ids=list(range(N_CORES)), trace=trace)
    out = np.empty((2, B, NOUT), np.float32)
    for c in range(N_CORES):
        yc = res.results[c]["y"]
        rows = slice(c * ROWS_PER_CORE, (c + 1) * ROWS_PER_CORE)
        out[0, rows] = yc[:ROWS_PER_CORE]
        out[1, rows] = yc[ROWS_PER_CORE:]
    return out, res


def kernel(x_real, x_imag, fir_filter, factor):
    assert int(factor) == FACTOR
    x_real = np.asarray(x_real, np.float32)
    x_imag = np.asarray(x_imag, np.float32)
    assert x_real.shape == (B, N) and x_imag.shape == (B, N)
    out, _ = _run(x_real, x_imag, fir_filter)
    return out
